# revision 9
# baseline (speedup 1.0000x reference)
"""Trainium2 Bass kernel for nn_CausalAttention_5815385719336.

Dual-softmax attention: out = softmax(-QK^T/8) V Wo^T (+bias folds),
out_comp = softmax(+QK^T/8) V Wo^T.  B=2, S=2048, D=1024, H=16, DK=64.

Sharding (8 cores): Megatron-style head parallel.  Core c owns heads
(2c, 2c+1) = output dims [128c, 128c+128) of the QKV projections.  Each
core computes its head slice of Q/K/V for both batches, the full [S,S]
attention for its 4 (b, head) units (both softmax branches), and a
partial output projection o_slice @ Wo_slice^T.  The host sums the 8
partial outputs and adds the bias fold (bv @ Wo^T + bo).

On-device dataflow is fully "transposed": the host ships x^T (and W^T
slices) so every matmul contracts along partitions with zero on-device
transposes.  Scores are built as scores^T [k, q]; exp runs on the
scalar engine straight out of PSUM; P^T @ V needs no transpose because
P^T is exactly what the PV matmul wants as its moving operand.  The
softmax denominator comes for free from a ones-column appended to V
(one extra PSUM row per head), is broadcast across partitions on
GPSIMD, reciprocated with the fast custom-DVE op, and folded into the
oT tiles before the output projection.
"""

import numpy as np
import ml_dtypes

B, S, D, H, DK = 2, 2048, 1024, 16, 64
NCORES = 8
HPC = H // NCORES          # heads per core = 2
DSL = HPC * DK             # d-slice per core = 128
P = 128
BF16 = ml_dtypes.bfloat16

_compiled = {}


def _install_drain_split():
    """walrus in this container rejects >1 sync wait on the Tile tail
    Drain; split extra waits into standalone wait_ge instructions."""
    import concourse.tile as tile
    from concourse.vector_clock import ScopedClock

    if getattr(tile.TileContext, "_drain_split_installed", False):
        return

    def _drain_and_barrier(self, tick_clock, wait_clock):
        nc = self.nc
        drain_inst = nc.sync.drain()
        wait_clock.add_sem_waits(
            drain_inst.ins, ScopedClock({None: tick_clock.global_clock})
        )
        si = drain_inst.ins.sync_info
        if si is not None and si.on_wait and len(si.on_wait) > 1:
            waits = list(si.on_wait)
            handles = {h.num: h for h in self.sems.allocated().values()}
            si.on_wait = waits[:1]
            for w in waits[1:]:
                assert w.wait_mode == "sem-ge-imm", w.wait_mode
                nc.sync.wait_ge(handles[w.id], w.wait_value)
        nc.all_engine_barrier()
        popped = nc._tile_sem_poison_stack.pop()
        assert popped is self._sem_poison
        nc.clear_and_free_semaphores(list(self.sems.allocated().values()))
        nc.all_engine_barrier()

    tile.TileContext._drain_and_barrier = _drain_and_barrier
    tile.TileContext._drain_split_installed = True


def _split_sync_waits(nc, max_waits=1):
    """walrus in this container has a small per-instruction sync-wait
    capacity.  Hoist excess waits onto standalone EventSemaphore
    instructions inserted just before the owner on the same engine —
    program order within an engine keeps the semantics identical."""
    from concourse import mybir

    n = 0
    for bb in nc.main_func.blocks:
        out = []
        for ins in bb.instructions:
            si = ins.sync_info
            if si is not None and si.on_wait and len(si.on_wait) > max_waits:
                waits = list(si.on_wait)
                for w in waits[:-max_waits]:
                    wi = mybir.InstEventSemaphore(name=f"W-split-{n}", ins=[], outs=[])
                    n += 1
                    wi.engine = ins.engine
                    wi.sync_info = mybir.SyncInfo(on_wait=[w], on_update=[])
                    out.append(wi)
                si.on_wait = waits[-max_waits:]
            out.append(ins)
        if n:
            bb.instructions = out


def _build():
    import concourse.bass as bass
    import concourse.tile as tile
    from concourse import mybir

    _install_drain_split()

    f32 = mybir.dt.float32
    bf16 = mybir.dt.bfloat16
    Exp = mybir.ActivationFunctionType.Exp
    Log = mybir.ActivationFunctionType.Ln
    NT = B * S                      # 4096 tokens
    ET = D // P                     # 8 e-tiles

    nc = bass.Bass()
    xt_d = nc.declare_dram_parameter("xt", [P, ET, NT], bf16, isOutput=False)
    wq_d = nc.declare_dram_parameter("wq", [P, ET, DSL], bf16, isOutput=False)
    wk_d = nc.declare_dram_parameter("wk", [P, ET, DSL], bf16, isOutput=False)
    wv_d = nc.declare_dram_parameter("wv", [P, ET, DSL], bf16, isOutput=False)
    wo_d = nc.declare_dram_parameter("wo", [64, HPC, D], bf16, isOutput=False)
    bq_d = nc.declare_dram_parameter("bq", [P, 1], f32, isOutput=False)
    bk_d = nc.declare_dram_parameter("bk", [P, 1], f32, isOutput=False)
    out_d = nc.declare_dram_parameter("out", [2, B, S, D], bf16, isOutput=True)

    KT = S // P                     # 16 k-tiles per batch
    TT = S // P                     # 16 token-tiles per batch
    QC = 2                          # q chunks of 1024
    QW = S // QC                    # 1024

    with tile.TileContext(nc) as tc:
        with (
            tc.tile_pool(name="singles", bufs=1) as singles,
            tc.tile_pool(name="perb", bufs=2) as perb,
            tc.tile_pool(name="expp", bufs=3) as expp,
            tc.tile_pool(name="normp", bufs=2) as normp,
            tc.tile_pool(name="outp", bufs=3) as outp,
            tc.tile_pool(name="ps_sc", bufs=2, space="PSUM") as ps_sc,
            tc.tile_pool(name="ps_acc", bufs=1, space="PSUM") as ps_acc,
            tc.tile_pool(name="ps_o", bufs=2, space="PSUM") as ps_o,
        ):
            xt = singles.tile([P, ET, NT], bf16)
            nc.sync.dma_start(xt[:], xt_d[:])
            wq = singles.tile([P, ET, DSL], bf16)
            nc.sync.dma_start(wq[:], wq_d[:])
            wk = singles.tile([P, ET, DSL], bf16)
            nc.sync.dma_start(wk[:], wk_d[:])
            wv = singles.tile([P, ET, DSL], bf16)
            nc.sync.dma_start(wv[:], wv_d[:])
            wo = singles.tile([64, HPC, D], bf16)
            nc.sync.dma_start(wo[:], wo_d[:])
            bq = singles.tile([P, 1], f32)
            nc.sync.dma_start(bq[:], bq_d[:])
            bk = singles.tile([P, 1], f32)
            nc.sync.dma_start(bk[:], bk_d[:])
            ones_sb = singles.tile([P, 64], bf16)
            nc.vector.memset(ones_sb[:], 1.0)

            for b in range(B):
                t0 = b * S
                qT = perb.tile([P, S], bf16, tag="qT")
                kT = perb.tile([P, S], bf16, tag="kT")
                # v layout per token-tile: cols 0:64 = V head0, col 64 =
                # ones, cols 65:129 = V head1, col 129 = ones.  Each head's
                # PV lhsT is [V | ones] -> PSUM rows 0:64 = oT, row 64 =
                # softmax denominator.  Both heads accumulate at PSUM base
                # partition 0 (matmul out base must be one of {0,32,64} and
                # bases 32/64 cap the partition span).
                vt = perb.tile([P, TT, 130], bf16, tag="vt")
                nc.vector.memset(vt[:, :, 64], 1.0)
                nc.vector.memset(vt[:, :, 129], 1.0)

                # Q^T, K^T projections: [d-slice on partitions, tokens free]
                for w_t, bias_t, dst in ((wq, bq, qT), (wk, bk, kT)):
                    for qc in range(4):
                        ps = ps_sc.tile([P, 512], f32, tag="sc")
                        for et in range(ET):
                            nc.tensor.matmul(
                                ps,
                                w_t[:, et, :],
                                xt[:, et, t0 + qc * 512 : t0 + (qc + 1) * 512],
                                start=(et == 0),
                                stop=(et == ET - 1),
                            )
                        nc.vector.tensor_scalar_add(
                            dst[:, qc * 512 : (qc + 1) * 512], ps, bias_t
                        )

                # V: natural layout [token partitions, d free]
                for tt in range(TT):
                    pv = ps_o.tile([P, DSL], f32, tag="po")
                    for et in range(ET):
                        nc.tensor.matmul(
                            pv,
                            xt[:, et, t0 + tt * P : t0 + (tt + 1) * P],
                            wv[:, et, :],
                            start=(et == 0),
                            stop=(et == ET - 1),
                        )
                    nc.vector.tensor_copy(vt[:, tt, 0:64], pv[:, 0:64])
                    nc.vector.tensor_copy(vt[:, tt, 65:129], pv[:, 64:128])

                # attention, branch 0 = exp(-s/8) -> 'out',
                #            branch 1 = exp(+s/8) -> 'out_comp'
                for br in range(2):
                    sgn = -0.125 if br == 0 else 0.125
                    oTs = [normp.tile([64, S], bf16, tag=f"oT{h}",
                                      name=f"oT{h}_{b}_{br}")
                           for h in range(HPC)]
                    for h in range(HPC):
                        hp = 64 * h
                        vlo, vhi = (0, 65) if h == 0 else (65, 130)
                        # denominators for both q-chunks, staged on row 64
                        dn = normp.tile([P, S], f32, tag="dn")
                        oTu = [normp.tile([64, QW], bf16, tag=f"oTu{qc}",
                                          name=f"oTu_{b}_{br}_{h}_{qc}")
                               for qc in range(QC)]
                        for qc in range(QC):
                            q0 = qc * QW
                            acc = ps_acc.tile([P, QW], f32, tag="acc")
                            # rows 0:64 = unnormalized oT, row 64 = denom
                            acc_ap = acc[0:65]
                            for kt in range(KT):
                                sc = ps_sc.tile([P, QW], f32, tag="sc")
                                for fh in range(2):
                                    nc.tensor.matmul(
                                        sc[:, fh * 512 : (fh + 1) * 512],
                                        kT[hp : hp + 64, kt * P : (kt + 1) * P],
                                        qT[hp : hp + 64,
                                           q0 + fh * 512 : q0 + (fh + 1) * 512],
                                        start=True,
                                        stop=True,
                                    )
                                ex = expp.tile([P, QW], bf16, tag="ex")
                                nc.scalar.activation(ex, sc, Exp, scale=sgn)
                                for fh in range(2):
                                    nc.tensor.matmul(
                                        acc_ap[:, fh * 512 : (fh + 1) * 512],
                                        vt[:, kt, vlo:vhi],
                                        ex[:, fh * 512 : (fh + 1) * 512],
                                        start=(kt == 0),
                                        stop=(kt == KT - 1),
                                    )
                            nc.vector.tensor_copy(
                                dn[64:65, q0 : q0 + QW], acc[64:65, :]
                            )
                            nc.vector.tensor_copy(oTu[qc][:], acc[0:64, :])
                        # 1/denom = exp(-Log(denom)): stays in the
                        # natural_log_exp ACT table set (no set switches)
                        lnd = normp.tile([P, S], f32, tag="lnd")
                        nc.scalar.activation(lnd[64:65, :], dn[64:65, :], Log)
                        rcp = normp.tile([P, S], bf16, tag="rcp")
                        nc.scalar.activation(
                            rcp[64:65, :], lnd[64:65, :], Exp, scale=-1.0
                        )
                        for qc in range(QC):
                            q0 = qc * QW
                            bc = ps_sc.tile([P, QW], f32, tag="sc")
                            for fh in range(2):
                                nc.tensor.matmul(
                                    bc[0:64, fh * 512 : (fh + 1) * 512],
                                    ones_sb[64:65, :],
                                    rcp[64:65,
                                        q0 + fh * 512 : q0 + (fh + 1) * 512],
                                    start=True,
                                    stop=True,
                                )
                            nc.vector.tensor_mul(
                                oTs[h][:, q0 : q0 + QW], oTu[qc][:], bc[0:64]
                            )
                    # output projection: out_partial = oT^T @ Wo_slice^T
                    for tt in range(TT):
                        ob = outp.tile([P, D], bf16, tag="ob")
                        for oc in range(2):
                            po = ps_o.tile([P, 512], f32, tag="po")
                            for h in range(HPC):
                                nc.tensor.matmul(
                                    po,
                                    oTs[h][:, tt * P : (tt + 1) * P],
                                    wo[:, h, oc * 512 : (oc + 1) * 512],
                                    start=(h == 0),
                                    stop=(h == HPC - 1),
                                )
                            nc.vector.tensor_copy(
                                ob[:, oc * 512 : (oc + 1) * 512], po
                            )
                        nc.sync.dma_start(
                            out_d[br, b, tt * P : (tt + 1) * P, :], ob[:]
                        )
    _split_sync_waits(nc)
    return nc


def _get_nc():
    if "nc" not in _compiled:
        _compiled["nc"] = _build()
    return _compiled["nc"]


def _prep_in_maps(x, Wq, bq, Wk, bk, Wv, bv, Wo, bo):
    ET = D // P
    xf = np.ascontiguousarray(x.reshape(B * S, D))
    # x^T tiled: [p, et, token], e = et*128 + p
    xt = np.ascontiguousarray(
        xf.T.reshape(ET, P, B * S).transpose(1, 0, 2)
    ).astype(BF16)
    in_maps = []
    for c in range(NCORES):
        sl = slice(DSL * c, DSL * (c + 1))
        wqt = np.ascontiguousarray(
            Wq[sl].T.reshape(ET, P, DSL).transpose(1, 0, 2)
        ).astype(BF16)
        wkt = np.ascontiguousarray(
            Wk[sl].T.reshape(ET, P, DSL).transpose(1, 0, 2)
        ).astype(BF16)
        wvt = np.ascontiguousarray(
            Wv[sl].T.reshape(ET, P, DSL).transpose(1, 0, 2)
        ).astype(BF16)
        # [64, h, dout]: row r, head h -> global d = 128*c + 64*h + r
        wot = np.ascontiguousarray(
            Wo[:, sl].T.reshape(HPC, 64, D).transpose(1, 0, 2)
        ).astype(BF16)
        in_maps.append(
            {
                "xt": xt,
                "wq": wqt,
                "wk": wkt,
                "wv": wvt,
                "wo": wot,
                "bq": np.ascontiguousarray(bq[sl].reshape(P, 1)).astype(np.float32),
                "bk": np.ascontiguousarray(bk[sl].reshape(P, 1)).astype(np.float32),
            }
        )
    return in_maps


def kernel(x, Wq, bq, Wk, bk, Wv, bv, Wo, bo, _trace=False, _tmpdir=None):
    from concourse.bass_utils import run_bass_kernel_spmd

    x, Wq, bq, Wk, bk, Wv, bv, Wo, bo = (
        np.asarray(a, dtype=np.float32)
        for a in (x, Wq, bq, Wk, bk, Wv, bv, Wo, bo)
    )
    nc = _get_nc()
    in_maps = _prep_in_maps(x, Wq, bq, Wk, bk, Wv, bv, Wo, bo)
    res = run_bass_kernel_spmd(
        nc, in_maps, core_ids=list(range(NCORES)), trace=_trace, tmpdir=_tmpdir
    )
    total = np.zeros((2, B, S, D), np.float32)
    for c in range(NCORES):
        total += np.asarray(res.results[c]["out"], dtype=np.float32)
    const_vec = (bv @ Wo.T + bo).astype(np.float32)
    out = total[0] + const_vec
    out_comp = total[1] + const_vec
    if _trace:
        kernel._last_result = res
    return (out, out_comp)


# revision 11
# speedup vs baseline: 1.1518x; 1.1518x over previous
"""Trainium2 Bass kernel for nn_CausalAttention_5815385719336.

Dual-softmax attention: out = softmax(-QK^T/8) V Wo^T (+bias folds),
out_comp = softmax(+QK^T/8) V Wo^T.  B=2, S=2048, D=1024, H=16, DK=64.

Sharding (8 cores): Megatron-style head parallel.  Core c owns heads
(2c, 2c+1) = output dims [128c, 128c+128) of the QKV projections.  Each
core computes its head slice of Q/K/V for both batches, the full [S,S]
attention for its 4 (b, head) units (both softmax branches), and a
partial output projection o_slice @ Wo_slice^T.  The host sums the 8
partial outputs and adds the bias fold (bv @ Wo^T + bo).

On-device dataflow is fully "transposed": the host ships x^T (and W^T
slices) so every matmul contracts along partitions with zero on-device
transposes.  Scores are built as scores^T [k, q]; exp runs on the
scalar engine straight out of PSUM; P^T @ V needs no transpose because
P^T is exactly what the PV matmul wants as its moving operand.  The
softmax denominator comes for free from a ones-column appended to V
(one extra PSUM row per head), is broadcast across partitions on
GPSIMD, reciprocated with the fast custom-DVE op, and folded into the
oT tiles before the output projection.
"""

import numpy as np
import ml_dtypes

B, S, D, H, DK = 2, 2048, 1024, 16, 64
NCORES = 8
HPC = H // NCORES          # heads per core = 2
DSL = HPC * DK             # d-slice per core = 128
P = 128
BF16 = ml_dtypes.bfloat16

_compiled = {}


def _install_drain_split():
    """walrus in this container rejects >1 sync wait on the Tile tail
    Drain; split extra waits into standalone wait_ge instructions."""
    import concourse.tile as tile
    from concourse.vector_clock import ScopedClock

    if getattr(tile.TileContext, "_drain_split_installed", False):
        return

    def _drain_and_barrier(self, tick_clock, wait_clock):
        nc = self.nc
        drain_inst = nc.sync.drain()
        wait_clock.add_sem_waits(
            drain_inst.ins, ScopedClock({None: tick_clock.global_clock})
        )
        si = drain_inst.ins.sync_info
        if si is not None and si.on_wait and len(si.on_wait) > 1:
            waits = list(si.on_wait)
            handles = {h.num: h for h in self.sems.allocated().values()}
            si.on_wait = waits[:1]
            for w in waits[1:]:
                assert w.wait_mode == "sem-ge-imm", w.wait_mode
                nc.sync.wait_ge(handles[w.id], w.wait_value)
        nc.all_engine_barrier()
        popped = nc._tile_sem_poison_stack.pop()
        assert popped is self._sem_poison
        nc.clear_and_free_semaphores(list(self.sems.allocated().values()))
        nc.all_engine_barrier()

    tile.TileContext._drain_and_barrier = _drain_and_barrier
    tile.TileContext._drain_split_installed = True


def _split_sync_waits(nc, max_waits=1):
    """walrus in this container has a small per-instruction sync-wait
    capacity.  Hoist excess waits onto standalone EventSemaphore
    instructions inserted just before the owner on the same engine —
    program order within an engine keeps the semantics identical."""
    from concourse import mybir

    n = 0
    for bb in nc.main_func.blocks:
        out = []
        for ins in bb.instructions:
            si = ins.sync_info
            if si is not None and si.on_wait and len(si.on_wait) > max_waits:
                waits = list(si.on_wait)
                for w in waits[:-max_waits]:
                    wi = mybir.InstEventSemaphore(name=f"W-split-{n}", ins=[], outs=[])
                    n += 1
                    wi.engine = ins.engine
                    wi.sync_info = mybir.SyncInfo(on_wait=[w], on_update=[])
                    out.append(wi)
                si.on_wait = waits[-max_waits:]
            out.append(ins)
        if n:
            bb.instructions = out


def _build():
    import concourse.bass as bass
    import concourse.tile as tile
    from concourse import mybir

    _install_drain_split()

    f32 = mybir.dt.float32
    bf16 = mybir.dt.bfloat16
    Exp = mybir.ActivationFunctionType.Exp
    Log = mybir.ActivationFunctionType.Ln
    NT = B * S                      # 4096 tokens
    ET = D // P                     # 8 e-tiles

    nc = bass.Bass()
    xt_d = nc.declare_dram_parameter("xt", [P, ET, NT], bf16, isOutput=False)
    wq_d = nc.declare_dram_parameter("wq", [P, ET, DSL], bf16, isOutput=False)
    wk_d = nc.declare_dram_parameter("wk", [P, ET, DSL], bf16, isOutput=False)
    wv_d = nc.declare_dram_parameter("wv", [P, ET, DSL], bf16, isOutput=False)
    wo_d = nc.declare_dram_parameter("wo", [64, HPC, D], bf16, isOutput=False)
    bq_d = nc.declare_dram_parameter("bq", [P, 1], f32, isOutput=False)
    bk_d = nc.declare_dram_parameter("bk", [P, 1], f32, isOutput=False)
    out_d = nc.declare_dram_parameter("out", [2, B, S, D], bf16, isOutput=True)

    KT = S // P                     # 16 k-tiles per batch
    TT = S // P                     # 16 token-tiles per batch
    QC = 2                          # q chunks of 1024
    QW = S // QC                    # 1024

    with tile.TileContext(nc) as tc:
        with (
            tc.tile_pool(name="singles", bufs=1) as singles,
            tc.tile_pool(name="xst", bufs=2) as xst,
            tc.tile_pool(name="perb", bufs=2) as perb,
            tc.tile_pool(name="stash", bufs=2) as stash,
            tc.tile_pool(name="expp", bufs=3) as expp,
            tc.tile_pool(name="otsp", bufs=1) as otsp,
            tc.tile_pool(name="normp", bufs=3) as normp,
            tc.tile_pool(name="outp", bufs=3) as outp,
            tc.tile_pool(name="ps_sc", bufs=2, space="PSUM") as ps_sc,
            tc.tile_pool(name="ps_acc", bufs=1, space="PSUM") as ps_acc,
            tc.tile_pool(name="ps_o", bufs=2, space="PSUM") as ps_o,
        ):
            wq = singles.tile([P, ET, DSL], bf16)
            nc.sync.dma_start(wq[:], wq_d[:])
            wk = singles.tile([P, ET, DSL], bf16)
            nc.sync.dma_start(wk[:], wk_d[:])
            wv = singles.tile([P, ET, DSL], bf16)
            nc.sync.dma_start(wv[:], wv_d[:])
            wo = singles.tile([64, HPC, D], bf16)
            nc.sync.dma_start(wo[:], wo_d[:])
            bq = singles.tile([P, 1], f32)
            nc.sync.dma_start(bq[:], bq_d[:])
            bk = singles.tile([P, 1], f32)
            nc.sync.dma_start(bk[:], bk_d[:])
            ones_sb = singles.tile([P, 64], bf16)
            nc.vector.memset(ones_sb[:], 1.0)

            for b in range(B):
                t0 = b * S
                qT = perb.tile([P, S], bf16, tag="qT")
                kT = perb.tile([P, S], bf16, tag="kT")
                # v layout per token-tile: cols 0:64 = V head0, col 64 =
                # ones, cols 65:129 = V head1, col 129 = ones.  Each head's
                # PV lhsT is [V | ones] -> PSUM rows 0:64 = oT, row 64 =
                # softmax denominator.  Both heads accumulate at PSUM base
                # partition 0 (matmul out base must be one of {0,32,64} and
                # bases 32/64 cap the partition span).
                vt = perb.tile([P, TT, 130], bf16, tag="vt")
                nc.vector.memset(vt[:, :, 64], 1.0)
                nc.vector.memset(vt[:, :, 129], 1.0)

                # Q^T, K^T, V projections off one streamed pass over x^T
                for qc in range(4):
                    xtile = xst.tile([P, ET, 512], bf16, tag="xtile")
                    nc.sync.dma_start(
                        xtile[:], xt_d[:, :, t0 + qc * 512 : t0 + (qc + 1) * 512]
                    )
                    for w_t, bias_t, dst in ((wq, bq, qT), (wk, bk, kT)):
                        ps = ps_sc.tile([P, 512], f32, tag="sc")
                        for et in range(ET):
                            nc.tensor.matmul(
                                ps,
                                w_t[:, et, :],
                                xtile[:, et, :],
                                start=(et == 0),
                                stop=(et == ET - 1),
                            )
                        nc.vector.tensor_scalar_add(
                            dst[:, qc * 512 : (qc + 1) * 512], ps, bias_t
                        )
                    for vtt in range(4):
                        tt = qc * 4 + vtt
                        pv = ps_o.tile([P, DSL], f32, tag="po")
                        for et in range(ET):
                            nc.tensor.matmul(
                                pv,
                                xtile[:, et, vtt * P : (vtt + 1) * P],
                                wv[:, et, :],
                                start=(et == 0),
                                stop=(et == ET - 1),
                            )
                        nc.vector.tensor_copy(vt[:, tt, 0:64], pv[:, 0:64])
                        nc.vector.tensor_copy(vt[:, tt, 65:129], pv[:, 64:128])

                # attention: scores computed ONCE per (h, qc, kt); both
                # softmax branches exp'd from the same PSUM tile.  Branch 0
                # (exp(-s/8) -> 'out') PV-accumulates inline; branch 1's
                # exp(+s/8) tiles are stashed in SBUF and consumed by a
                # dense PV-only second pass.
                oTs = [[otsp.tile([64, S], bf16, tag=f"oT{h}_{br}",
                                   name=f"oT{h}_{br}_{b}")
                        for h in range(HPC)] for br in range(2)]
                for h in range(HPC):
                    hp = 64 * h
                    vlo, vhi = (0, 65) if h == 0 else (65, 130)
                    for qc in range(QC):
                        q0 = qc * QW
                        exn = stash.tile([P, KT, QW], bf16, tag="exn")
                        accs = []
                        for br in range(2):
                            acc = ps_acc.tile([P, QW], f32, tag="acc",
                                              name=f"acc_{b}_{h}_{qc}_{br}")
                            if br == 0:
                                for kt in range(KT):
                                    sc = ps_sc.tile([P, QW], f32, tag="sc")
                                    for fh in range(2):
                                        nc.tensor.matmul(
                                            sc[:, fh * 512 : (fh + 1) * 512],
                                            kT[hp : hp + 64,
                                               kt * P : (kt + 1) * P],
                                            qT[hp : hp + 64,
                                               q0 + fh * 512 :
                                               q0 + (fh + 1) * 512],
                                            start=True,
                                            stop=True,
                                        )
                                    ex = expp.tile([P, QW], bf16, tag="ex")
                                    nc.scalar.activation(
                                        ex, sc, Exp, scale=-0.125
                                    )
                                    nc.scalar.activation(
                                        exn[:, kt, :], sc, Exp, scale=0.125
                                    )
                                    for fh in range(2):
                                        nc.tensor.matmul(
                                            acc[0:65,
                                                fh * 512 : (fh + 1) * 512],
                                            vt[:, kt, vlo:vhi],
                                            ex[:, fh * 512 : (fh + 1) * 512],
                                            start=(kt == 0),
                                            stop=(kt == KT - 1),
                                        )
                            else:
                                for kt in range(KT):
                                    for fh in range(2):
                                        nc.tensor.matmul(
                                            acc[0:65,
                                                fh * 512 : (fh + 1) * 512],
                                            vt[:, kt, vlo:vhi],
                                            exn[:, kt,
                                                fh * 512 : (fh + 1) * 512],
                                            start=(kt == 0),
                                            stop=(kt == KT - 1),
                                        )
                            # 1/denom = exp(-Ln(denom)) on the scalar engine:
                            # same natural_log_exp table set as the main exps
                            lnd = normp.tile([P, QW], f32, tag="lnd")
                            nc.scalar.activation(
                                lnd[64:65, :], acc[64:65, :], Log
                            )
                            rcp = normp.tile([P, QW], bf16, tag="rcp")
                            nc.scalar.activation(
                                rcp[64:65, :], lnd[64:65, :], Exp, scale=-1.0
                            )
                            oTu = normp.tile([64, QW], bf16, tag="oTu",
                                             name=f"oTu_{b}_{h}_{qc}_{br}")
                            nc.vector.tensor_copy(oTu[:], acc[0:64, :])
                            bc = ps_sc.tile([P, QW], f32, tag="sc")
                            for fh in range(2):
                                nc.tensor.matmul(
                                    bc[0:64, fh * 512 : (fh + 1) * 512],
                                    ones_sb[64:65, :],
                                    rcp[64:65, fh * 512 : (fh + 1) * 512],
                                    start=True,
                                    stop=True,
                                )
                            nc.vector.tensor_mul(
                                oTs[br][h][:, q0 : q0 + QW], oTu[:], bc[0:64]
                            )
                # output projection: out_partial = oT^T @ Wo_slice^T
                for br in range(2):
                    for tt in range(TT):
                        ob = outp.tile([P, D], bf16, tag="ob")
                        for oc in range(2):
                            po = ps_o.tile([P, 512], f32, tag="po")
                            for h in range(HPC):
                                nc.tensor.matmul(
                                    po,
                                    oTs[br][h][:, tt * P : (tt + 1) * P],
                                    wo[:, h, oc * 512 : (oc + 1) * 512],
                                    start=(h == 0),
                                    stop=(h == HPC - 1),
                                )
                            nc.vector.tensor_copy(
                                ob[:, oc * 512 : (oc + 1) * 512], po
                            )
                        nc.sync.dma_start(
                            out_d[br, b, tt * P : (tt + 1) * P, :], ob[:]
                        )
    _split_sync_waits(nc)
    return nc


def _get_nc():
    if "nc" not in _compiled:
        _compiled["nc"] = _build()
    return _compiled["nc"]


def _prep_in_maps(x, Wq, bq, Wk, bk, Wv, bv, Wo, bo):
    ET = D // P
    xf = np.ascontiguousarray(x.reshape(B * S, D))
    # x^T tiled: [p, et, token], e = et*128 + p
    xt = np.ascontiguousarray(
        xf.T.reshape(ET, P, B * S).transpose(1, 0, 2)
    ).astype(BF16)
    in_maps = []
    for c in range(NCORES):
        sl = slice(DSL * c, DSL * (c + 1))
        wqt = np.ascontiguousarray(
            Wq[sl].T.reshape(ET, P, DSL).transpose(1, 0, 2)
        ).astype(BF16)
        wkt = np.ascontiguousarray(
            Wk[sl].T.reshape(ET, P, DSL).transpose(1, 0, 2)
        ).astype(BF16)
        wvt = np.ascontiguousarray(
            Wv[sl].T.reshape(ET, P, DSL).transpose(1, 0, 2)
        ).astype(BF16)
        # [64, h, dout]: row r, head h -> global d = 128*c + 64*h + r
        wot = np.ascontiguousarray(
            Wo[:, sl].T.reshape(HPC, 64, D).transpose(1, 0, 2)
        ).astype(BF16)
        in_maps.append(
            {
                "xt": xt,
                "wq": wqt,
                "wk": wkt,
                "wv": wvt,
                "wo": wot,
                "bq": np.ascontiguousarray(bq[sl].reshape(P, 1)).astype(np.float32),
                "bk": np.ascontiguousarray(bk[sl].reshape(P, 1)).astype(np.float32),
            }
        )
    return in_maps


def kernel(x, Wq, bq, Wk, bk, Wv, bv, Wo, bo, _trace=False, _tmpdir=None):
    from concourse.bass_utils import run_bass_kernel_spmd

    x, Wq, bq, Wk, bk, Wv, bv, Wo, bo = (
        np.asarray(a, dtype=np.float32)
        for a in (x, Wq, bq, Wk, bk, Wv, bv, Wo, bo)
    )
    nc = _get_nc()
    in_maps = _prep_in_maps(x, Wq, bq, Wk, bk, Wv, bv, Wo, bo)
    res = run_bass_kernel_spmd(
        nc, in_maps, core_ids=list(range(NCORES)), trace=_trace, tmpdir=_tmpdir
    )
    total = np.zeros((2, B, S, D), np.float32)
    for c in range(NCORES):
        total += np.asarray(res.results[c]["out"], dtype=np.float32)
    const_vec = (bv @ Wo.T + bo).astype(np.float32)
    out = total[0] + const_vec
    out_comp = total[1] + const_vec
    if _trace:
        kernel._last_result = res
    return (out, out_comp)
